# revision 20
# baseline (speedup 1.0000x reference)
"""Int4 tensor-parallel linear for TRN2 (8 NeuronCores), fp8-hybrid version.

out[B,S,N] = x[B,S,K] @ dequant(weight_packed, scales).T + bias

Sharding: weight_packed/scales/bias split along N (11008 -> 8 x 1376);
x replicated. Each core computes out[:, n_shard]; host concatenates.

Per-core kernel:
- Host repacks weight_packed to [KH, NSH] (contraction dim on SBUF
  partitions), so dequant lands directly in matmul-ready wT[k, n] layout
  with NO PE transposes. Nibble order is absorbed by permuting xT rows
  on the host (contraction order is free).
- Scales are host-pre-broadcast to [128, NSH] per kh-tile (x1024 so fp8
  weights sit in e4m3's normal range); dequant is 2 DVE ops per nibble
  plane: u8 extract, then fused (q - 8) * s via scalar_tensor_tensor.
- Hybrid precision: first K8 of the (permuted) contraction in fp8e4
  using DoubleRow matmuls (2 k-tiles per instruction, 2x PE rate), the
  rest in fp16. K8=1024 keeps rel err ~1.9e-2 < 2e-2.
- Output: single fused DVE pass (psum * 1/1024 + bias) -> fp16 -> DMA.
"""

import sys

if "/opt/trn_rl_repo" not in sys.path:
    sys.path.insert(0, "/opt/trn_rl_repo")

from contextlib import ExitStack

import numpy as np
import ml_dtypes

import concourse.bass as bass
import concourse.bacc as bacc
import concourse.mybir as mybir
import concourse.tile as tile
from concourse.bass_utils import run_bass_kernel_spmd

F16 = mybir.dt.float16
F32 = mybir.dt.float32
F8 = mybir.dt.float8e4
U8 = mybir.dt.uint8
E4 = ml_dtypes.float8_e4m3

B, S, K, N = 4, 1024, 4096, 11008
T = B * S
NCORES = 8
NSH = N // NCORES
KH = K // 2
NKH = KH // 128  # 16 kh-tiles

WSCALE = 1024.0  # pow2 lift of w into e4m3 normal range (exact)


def build_kernel(K8=1024, TB=512, xt16_bufs=48, psum_bufs=8, chunk_w=(512, 512, 352)):
    assert K8 % 256 == 0 and T % TB == 0 and TB % 128 == 0
    NP8 = K8 // 256            # DoubleRow pair tiles (kh-tiles 0..NP8-1)
    NT16 = (K - K8) // 128     # fp16 k-tiles
    assert sum(chunk_w) == NSH
    chunks = []
    c0 = 0
    for w in chunk_w:
        chunks.append((c0, w))
        c0 += w

    nc = bacc.Bacc("TRN2", target_bir_lowering=False, debug=False)
    xt8_d = nc.dram_tensor("xt8", (K8, T), F8, kind="ExternalInput")
    xt16_d = nc.dram_tensor("xt16", (K - K8, T), F16, kind="ExternalInput")
    wpT_d = nc.dram_tensor("wpT", (KH, NSH), U8, kind="ExternalInput")
    sbc_d = nc.dram_tensor("sbc", (NKH, 128, NSH), F16, kind="ExternalInput")
    bias_d = nc.dram_tensor("bias", (1, NSH), F16, kind="ExternalInput")
    out_d = nc.dram_tensor("out", (T, NSH), F16, kind="ExternalOutput")

    with tile.TileContext(nc) as tc, ExitStack() as ctx:
        const_p = ctx.enter_context(tc.tile_pool(name="const", bufs=1))
        w8_p = ctx.enter_context(tc.tile_pool(name="w8", bufs=1))
        w16_p = ctx.enter_context(tc.tile_pool(name="w16", bufs=1))
        wp_p = ctx.enter_context(tc.tile_pool(name="wpk", bufs=2))
        sbc_p = ctx.enter_context(tc.tile_pool(name="sbc", bufs=6))
        q_p = ctx.enter_context(tc.tile_pool(name="q", bufs=6))
        qf_p = ctx.enter_context(tc.tile_pool(name="qf", bufs=6))
        xt16_p = ctx.enter_context(tc.tile_pool(name="xt16", bufs=xt16_bufs))
        xt8_p = ctx.enter_context(tc.tile_pool(name="xt8", bufs=6))
        ob_p = ctx.enter_context(tc.tile_pool(name="ob", bufs=4))
        mpsum = ctx.enter_context(
            tc.tile_pool(name="mpsum", bufs=psum_bufs, space="PSUM")
        )

        # bias broadcast [128, NSH] via ones matmul
        bias_row = const_p.tile([1, NSH], F16)
        nc.sync.dma_start(bias_row[:], bias_d[:, :])
        ones_row = const_p.tile([1, 128], F16)
        nc.vector.memset(ones_row[:], 1.0)
        m8 = const_p.tile([128, 1], F32)
        nc.vector.memset(m8[:], -8.0)
        bias_b = const_p.tile([128, NSH], F16)
        for c0, csz in chunks:
            bp = mpsum.tile([128, 512], F32, tag="mp", name=f"biasb{c0}")
            nc.tensor.matmul(
                bp[:, :csz], ones_row[:], bias_row[:, c0 : c0 + csz],
                start=True, stop=True,
            )
            nc.vector.tensor_copy(bias_b[:, c0 : c0 + csz], bp[:, :csz])

        # persistent dequantized weights
        wT8 = [w8_p.tile([128, 2, NSH], F8, tag=f"w8_{j}", name=f"w8_{j}")
               for j in range(NP8)]
        wT16 = [w16_p.tile([128, NSH], F16, tag=f"w16_{m}", name=f"w16_{m}")
                for m in range(NT16)]

        x8_cache, x16_cache = {}, {}

        def x8_for(tb):
            if tb not in x8_cache:
                t0 = tb * TB
                x8s = []
                for jp in range(NP8):
                    x8_t = xt8_p.tile([128, 2, TB], F8, tag="x8")
                    nc.sync.dma_start(
                        x8_t[:],
                        xt8_d[jp * 256 : (jp + 1) * 256, t0 : t0 + TB].rearrange(
                            "(i p) t -> p i t", i=2
                        ),
                    )
                    x8s.append(x8_t)
                x8_cache[tb] = x8s
            return x8_cache[tb]

        def x16_for(tb):
            if tb not in x16_cache:
                t0 = tb * TB
                x16s = []
                for m in range(NT16):
                    x16_t = xt16_p.tile([128, TB], F16, tag="x16")
                    nc.sync.dma_start(
                        x16_t[:], xt16_d[m * 128 : (m + 1) * 128, t0 : t0 + TB]
                    )
                    x16s.append(x16_t)
                x16_cache[tb] = x16s
            return x16_cache[tb]

        # ---- dequant: per kh-tile, 2 DVE ops per nibble plane ----
        for j in range(NKH):
            wp_t = wp_p.tile([128, NSH], U8, tag="wp")
            nc.sync.dma_start(wp_t[:], wpT_d[j * 128 : (j + 1) * 128, :])
            sbc_t = sbc_p.tile([128, NSH], F16, tag="sbc")
            nc.sync.dma_start(sbc_t[:], sbc_d[j, :, :])
            q_lo8 = q_p.tile([128, NSH], U8, tag="qlo8")
            q_hi8 = q_p.tile([128, NSH], U8, tag="qhi8")
            nc.vector.tensor_scalar(
                q_lo8[:], wp_t[:], 15, None, op0=mybir.AluOpType.bitwise_and
            )
            nc.vector.tensor_scalar(
                q_hi8[:], wp_t[:], 4, None,
                op0=mybir.AluOpType.logical_shift_right,
            )
            # u8 -> f16 cast (with the -8 offset folded in) on the otherwise
            # idle ACT engine, so the scale multiply below runs with
            # all-16-bit SBUF operands (DVE fast path)
            q_lo = qf_p.tile([128, NSH], F16, tag="qlof")
            q_hi = qf_p.tile([128, NSH], F16, tag="qhif")
            nc.scalar.activation(q_lo[:], q_lo8[:],
                                 mybir.ActivationFunctionType.Identity,
                                 bias=m8[:])
            nc.scalar.activation(q_hi[:], q_hi8[:],
                                 mybir.ActivationFunctionType.Identity,
                                 bias=m8[:])
            if j < NP8:
                outs = (wT8[j][:, 0, :], wT8[j][:, 1, :])
            else:
                outs = (wT16[2 * (j - NP8)][:], wT16[2 * (j - NP8) + 1][:])
            nc.vector.tensor_tensor(outs[0], q_lo[:], sbc_t[:],
                                    op=mybir.AluOpType.mult)
            nc.vector.tensor_tensor(outs[1], q_hi[:], sbc_t[:],
                                    op=mybir.AluOpType.mult)
            if j == 0:
                # slot tb0's small fp8 x-tile DMAs (0.5 MB) right behind
                # kh-tile 0's transfers: early enough for the first
                # DoubleRow matmuls, without delaying the weight stream
                x8_for(0)

        # ---- matmul: groups of 8 psum tiles, k-major within a group ----
        # Interleaving the contraction across 8 concurrent psum tiles means
        # the PE always has ~8 ready instructions per dequantized kh-tile
        # during the startup window, instead of blocking in-order inside a
        # single 28-instruction chain waiting for the last kh-tile.
        DR = mybir.MatmulPerfMode.DoubleRow
        items = [
            (tb, ci, ts)
            for tb in range(T // TB)
            for ci in range(len(chunks))
            for ts in range(TB // 128)
        ]
        GRP = 8
        for g0 in range(0, len(items), GRP):
            grp = items[g0 : g0 + GRP]
            pss = []
            for tb, ci, ts in grp:
                x8_for(tb)
                x16_for(tb)
                pss.append(
                    mpsum.tile([128, 512], F32, tag="mp", name=f"mp{tb}_{ci}_{ts}")
                )
            for jp in range(NP8):
                for pi, (tb, ci, ts) in enumerate(grp):
                    c0, csz = chunks[ci]
                    nc.tensor.matmul(
                        pss[pi][:, :csz],
                        x8_for(tb)[jp][:, :, ts * 128 : (ts + 1) * 128],
                        wT8[jp][:, :, c0 : c0 + csz],
                        perf_mode=DR,
                        start=(jp == 0), stop=False,
                    )
            for m in range(NT16):
                for pi, (tb, ci, ts) in enumerate(grp):
                    c0, csz = chunks[ci]
                    nc.tensor.matmul(
                        pss[pi][:, :csz],
                        x16_for(tb)[m][:, ts * 128 : (ts + 1) * 128],
                        wT16[m][:, c0 : c0 + csz],
                        start=False, stop=(m == NT16 - 1),
                    )
            for pi, (tb, ci, ts) in enumerate(grp):
                c0, csz = chunks[ci]
                ob = ob_p.tile([128, 512], F16, tag="ob", name=f"ob{tb}_{ci}_{ts}")
                nc.vector.scalar_tensor_tensor(
                    ob[:, :csz], pss[pi][:, :csz], 1.0 / WSCALE,
                    bias_b[:, c0 : c0 + csz],
                    op0=mybir.AluOpType.mult, op1=mybir.AluOpType.add,
                )
                row0 = tb * TB + ts * 128
                nc.sync.dma_start(
                    out_d[row0 : row0 + 128, c0 : c0 + csz], ob[:, :csz]
                )

    nc.compile()
    return nc


_NC_CACHE = {}


def _get_nc(**kw):
    key = tuple(sorted(kw.items()))
    if key not in _NC_CACHE:
        _NC_CACHE[key] = build_kernel(**kw)
    return _NC_CACHE[key]


def _korder():
    # kh-tile j contributes k-tiles {2*kh} (low nibble) then {2*kh+1}
    kh = np.arange(KH).reshape(NKH, 128)
    return np.concatenate([2 * kh, 2 * kh + 1], axis=1).reshape(-1)


def _prep_in_maps(x, weight_packed, scales, bias, K8):
    x = np.asarray(x, dtype=np.float16)
    wp = np.asarray(weight_packed)
    if wp.dtype != np.uint8:
        wp = wp.astype(np.uint8)
    sc = np.asarray(scales, dtype=np.float16)
    b = np.asarray(bias, dtype=np.float16).reshape(1, N)

    xT = x.reshape(T, K).T[_korder()]  # [K, T], permuted contraction order
    xt8 = np.clip(xT[:K8].astype(np.float32), -240, 240).astype(E4)
    xt16 = np.ascontiguousarray(xT[K8:])

    # sbc[j, p, n] = 1024 * sc[n, 2j + (p>=64)]  (kh-tile j: first 64
    # partitions are scale group 2j, last 64 are group 2j+1)
    sc1024 = (sc.astype(np.float32) * WSCALE).astype(np.float16)  # [N, G]
    in_maps = []
    for c in range(NCORES):
        sl = slice(c * NSH, (c + 1) * NSH)
        scc = sc1024[sl]  # [NSH, G]
        sbc = np.empty((NKH, 128, NSH), np.float16)
        for j in range(NKH):
            sbc[j, :64] = scc[:, 2 * j]
            sbc[j, 64:] = scc[:, 2 * j + 1]
        in_maps.append(
            {
                "xt8": xt8,
                "xt16": xt16,
                "wpT": np.ascontiguousarray(wp[sl].T),
                "sbc": sbc,
                "bias": np.ascontiguousarray(b[:, sl]),
            }
        )
    return in_maps


def run(x, weight_packed, scales, bias, trace=False, **build_kw):
    nc = _get_nc(**build_kw)
    K8 = build_kw.get("K8", 1024)
    in_maps = _prep_in_maps(x, weight_packed, scales, bias, K8)
    res = run_bass_kernel_spmd(
        nc, in_maps, core_ids=list(range(NCORES)), trace=trace
    )
    out = np.concatenate([r["out"] for r in res.results], axis=1)
    return out.reshape(B, S, N), res


def kernel(x, weight_packed, scales, bias, group_size=128, **_ignored):
    assert int(np.asarray(group_size)) == 128
    out, _ = run(x, weight_packed, scales, bias)
    return out


# revision 21
# speedup vs baseline: 1.0119x; 1.0119x over previous
"""Int4 tensor-parallel linear for TRN2 (8 NeuronCores), fp8-hybrid version.

out[B,S,N] = x[B,S,K] @ dequant(weight_packed, scales).T + bias

Sharding: weight_packed/scales/bias split along N (11008 -> 8 x 1376);
x replicated. Each core computes out[:, n_shard]; host concatenates.

Per-core kernel:
- Host repacks weight_packed to [KH, NSH] (contraction dim on SBUF
  partitions), so dequant lands directly in matmul-ready wT[k, n] layout
  with NO PE transposes. Nibble order is absorbed by permuting xT rows
  on the host (contraction order is free).
- Scales are host-pre-broadcast to [128, NSH] per kh-tile (x1024 so fp8
  weights sit in e4m3's normal range); dequant is 2 DVE ops per nibble
  plane: u8 extract, then fused (q - 8) * s via scalar_tensor_tensor.
- Hybrid precision: first K8 of the (permuted) contraction in fp8e4
  using DoubleRow matmuls (2 k-tiles per instruction, 2x PE rate), the
  rest in fp16. K8=1024 keeps rel err ~1.9e-2 < 2e-2.
- Output: single fused DVE pass (psum * 1/1024 + bias) -> fp16 -> DMA.
"""

import sys

if "/opt/trn_rl_repo" not in sys.path:
    sys.path.insert(0, "/opt/trn_rl_repo")

from contextlib import ExitStack

import numpy as np
import ml_dtypes

import concourse.bass as bass
import concourse.bacc as bacc
import concourse.mybir as mybir
import concourse.tile as tile
from concourse.bass_utils import run_bass_kernel_spmd

F16 = mybir.dt.float16
F32 = mybir.dt.float32
F8 = mybir.dt.float8e4
U8 = mybir.dt.uint8
E4 = ml_dtypes.float8_e4m3

B, S, K, N = 4, 1024, 4096, 11008
T = B * S
NCORES = 8
NSH = N // NCORES
KH = K // 2
NKH = KH // 128  # 16 kh-tiles

WSCALE = 1024.0  # pow2 lift of w into e4m3 normal range (exact)


def build_kernel(K8=1024, TB=512, xt16_bufs=48, psum_bufs=8, chunk_w=(512, 512, 352)):
    assert K8 % 256 == 0 and T % TB == 0 and TB % 128 == 0
    NP8 = K8 // 256            # DoubleRow pair tiles (kh-tiles 0..NP8-1)
    NT16 = (K - K8) // 128     # fp16 k-tiles
    assert sum(chunk_w) == NSH
    chunks = []
    c0 = 0
    for w in chunk_w:
        chunks.append((c0, w))
        c0 += w

    nc = bacc.Bacc("TRN2", target_bir_lowering=False, debug=False)
    xt8_d = nc.dram_tensor("xt8", (K8, T), F8, kind="ExternalInput")
    xt16_d = nc.dram_tensor("xt16", (K - K8, T), F16, kind="ExternalInput")
    wpT_d = nc.dram_tensor("wpT", (KH, NSH), U8, kind="ExternalInput")
    sbc_d = nc.dram_tensor("sbc", (NKH, 128, NSH), F16, kind="ExternalInput")
    bias_d = nc.dram_tensor("bias", (1, NSH), F16, kind="ExternalInput")
    out_d = nc.dram_tensor("out", (T, NSH), F16, kind="ExternalOutput")

    with tile.TileContext(nc) as tc, ExitStack() as ctx:
        const_p = ctx.enter_context(tc.tile_pool(name="const", bufs=1))
        w8_p = ctx.enter_context(tc.tile_pool(name="w8", bufs=1))
        w16_p = ctx.enter_context(tc.tile_pool(name="w16", bufs=1))
        wp_p = ctx.enter_context(tc.tile_pool(name="wpk", bufs=2))
        sbc_p = ctx.enter_context(tc.tile_pool(name="sbc", bufs=6))
        q_p = ctx.enter_context(tc.tile_pool(name="q", bufs=6))
        qf_p = ctx.enter_context(tc.tile_pool(name="qf", bufs=6))
        xt16_p = ctx.enter_context(tc.tile_pool(name="xt16", bufs=xt16_bufs))
        xt8_p = ctx.enter_context(tc.tile_pool(name="xt8", bufs=6))
        ob_p = ctx.enter_context(tc.tile_pool(name="ob", bufs=4))
        mpsum = ctx.enter_context(
            tc.tile_pool(name="mpsum", bufs=psum_bufs, space="PSUM")
        )

        # bias broadcast [128, NSH] via ones matmul
        bias_row = const_p.tile([1, NSH], F16)
        nc.sync.dma_start(bias_row[:], bias_d[:, :])
        ones_row = const_p.tile([1, 128], F16)
        nc.vector.memset(ones_row[:], 1.0)
        m8 = const_p.tile([128, 1], F32)
        nc.vector.memset(m8[:], -8.0)
        bias_b = const_p.tile([128, NSH], F16)
        for c0, csz in chunks:
            bp = mpsum.tile([128, 512], F32, tag="mp", name=f"biasb{c0}")
            nc.tensor.matmul(
                bp[:, :csz], ones_row[:], bias_row[:, c0 : c0 + csz],
                start=True, stop=True,
            )
            nc.vector.tensor_copy(bias_b[:, c0 : c0 + csz], bp[:, :csz])

        # persistent dequantized weights
        wT8 = [w8_p.tile([128, 2, NSH], F8, tag=f"w8_{j}", name=f"w8_{j}")
               for j in range(NP8)]
        wT16 = [w16_p.tile([128, NSH], F16, tag=f"w16_{m}", name=f"w16_{m}")
                for m in range(NT16)]

        x8_cache, x16_cache = {}, {}

        def x8_for(tb):
            if tb not in x8_cache:
                t0 = tb * TB
                x8s = []
                for jp in range(NP8):
                    x8_t = xt8_p.tile([128, 2, TB], F8, tag="x8")
                    nc.sync.dma_start(
                        x8_t[:],
                        xt8_d[jp * 256 : (jp + 1) * 256, t0 : t0 + TB].rearrange(
                            "(i p) t -> p i t", i=2
                        ),
                    )
                    x8s.append(x8_t)
                x8_cache[tb] = x8s
            return x8_cache[tb]

        def x16_for(tb):
            if tb not in x16_cache:
                t0 = tb * TB
                x16s = []
                for m in range(NT16):
                    x16_t = xt16_p.tile([128, TB], F16, tag="x16")
                    nc.sync.dma_start(
                        x16_t[:], xt16_d[m * 128 : (m + 1) * 128, t0 : t0 + TB]
                    )
                    x16s.append(x16_t)
                x16_cache[tb] = x16s
            return x16_cache[tb]

        # ---- dequant: per kh-tile, 2 DVE ops per nibble plane ----
        for j in range(NKH):
            wp_t = wp_p.tile([128, NSH], U8, tag="wp")
            nc.sync.dma_start(wp_t[:], wpT_d[j * 128 : (j + 1) * 128, :])
            sbc_t = sbc_p.tile([128, NSH], F16, tag="sbc")
            nc.sync.dma_start(sbc_t[:], sbc_d[j, :, :])
            q_lo8 = q_p.tile([128, NSH], U8, tag="qlo8")
            q_hi8 = q_p.tile([128, NSH], U8, tag="qhi8")
            nc.vector.tensor_scalar(
                q_lo8[:], wp_t[:], 15, None, op0=mybir.AluOpType.bitwise_and
            )
            nc.vector.tensor_scalar(
                q_hi8[:], wp_t[:], 4, None,
                op0=mybir.AluOpType.logical_shift_right,
            )
            # u8 -> f16 cast (with the -8 offset folded in) on the otherwise
            # idle ACT engine, so the scale multiply below runs with
            # all-16-bit SBUF operands (DVE fast path)
            q_lo = qf_p.tile([128, NSH], F16, tag="qlof")
            q_hi = qf_p.tile([128, NSH], F16, tag="qhif")
            nc.scalar.activation(q_lo[:], q_lo8[:],
                                 mybir.ActivationFunctionType.Identity,
                                 bias=m8[:])
            nc.scalar.activation(q_hi[:], q_hi8[:],
                                 mybir.ActivationFunctionType.Identity,
                                 bias=m8[:])
            if j < NP8:
                outs = (wT8[j][:, 0, :], wT8[j][:, 1, :])
            else:
                outs = (wT16[2 * (j - NP8)][:], wT16[2 * (j - NP8) + 1][:])
            nc.vector.tensor_tensor(outs[0], q_lo[:], sbc_t[:],
                                    op=mybir.AluOpType.mult)
            nc.vector.tensor_tensor(outs[1], q_hi[:], sbc_t[:],
                                    op=mybir.AluOpType.mult)

        # ---- matmul: groups of 8 psum tiles, k-major within a group ----
        # Interleaving the contraction across 8 concurrent psum tiles means
        # the PE always has ~8 ready instructions per dequantized kh-tile
        # during the startup window, instead of blocking in-order inside a
        # single 28-instruction chain waiting for the last kh-tile.
        DR = mybir.MatmulPerfMode.DoubleRow
        items = [
            (tb, ci, ts)
            for tb in range(T // TB)
            for ci in range(len(chunks))
            for ts in range(TB // 128)
        ]
        GRP = 8
        for g0 in range(0, len(items), GRP):
            grp = items[g0 : g0 + GRP]
            pss = []
            for tb, ci, ts in grp:
                x8_for(tb)
                x16_for(tb)
                pss.append(
                    mpsum.tile([128, 512], F32, tag="mp", name=f"mp{tb}_{ci}_{ts}")
                )
            for jp in range(NP8):
                for pi, (tb, ci, ts) in enumerate(grp):
                    c0, csz = chunks[ci]
                    nc.tensor.matmul(
                        pss[pi][:, :csz],
                        x8_for(tb)[jp][:, :, ts * 128 : (ts + 1) * 128],
                        wT8[jp][:, :, c0 : c0 + csz],
                        perf_mode=DR,
                        start=(jp == 0), stop=False,
                    )
            for m in range(NT16):
                for pi, (tb, ci, ts) in enumerate(grp):
                    c0, csz = chunks[ci]
                    nc.tensor.matmul(
                        pss[pi][:, :csz],
                        x16_for(tb)[m][:, ts * 128 : (ts + 1) * 128],
                        wT16[m][:, c0 : c0 + csz],
                        start=False, stop=(m == NT16 - 1),
                    )
            for pi, (tb, ci, ts) in enumerate(grp):
                c0, csz = chunks[ci]
                ob = ob_p.tile([128, 512], F16, tag="ob", name=f"ob{tb}_{ci}_{ts}")
                nc.vector.scalar_tensor_tensor(
                    ob[:, :csz], pss[pi][:, :csz], 1.0 / WSCALE,
                    bias_b[:, c0 : c0 + csz],
                    op0=mybir.AluOpType.mult, op1=mybir.AluOpType.add,
                )
                row0 = tb * TB + ts * 128
                nc.sync.dma_start(
                    out_d[row0 : row0 + 128, c0 : c0 + csz], ob[:, :csz]
                )

    nc.compile()
    return nc


_NC_CACHE = {}


def _get_nc(**kw):
    key = tuple(sorted(kw.items()))
    if key not in _NC_CACHE:
        _NC_CACHE[key] = build_kernel(**kw)
    return _NC_CACHE[key]


def _korder():
    # kh-tile j contributes k-tiles {2*kh} (low nibble) then {2*kh+1}
    kh = np.arange(KH).reshape(NKH, 128)
    return np.concatenate([2 * kh, 2 * kh + 1], axis=1).reshape(-1)


def _prep_in_maps(x, weight_packed, scales, bias, K8):
    x = np.asarray(x, dtype=np.float16)
    wp = np.asarray(weight_packed)
    if wp.dtype != np.uint8:
        wp = wp.astype(np.uint8)
    sc = np.asarray(scales, dtype=np.float16)
    b = np.asarray(bias, dtype=np.float16).reshape(1, N)

    xT = x.reshape(T, K).T[_korder()]  # [K, T], permuted contraction order
    xt8 = np.clip(xT[:K8].astype(np.float32), -240, 240).astype(E4)
    xt16 = np.ascontiguousarray(xT[K8:])

    # sbc[j, p, n] = 1024 * sc[n, 2j + (p>=64)]  (kh-tile j: first 64
    # partitions are scale group 2j, last 64 are group 2j+1)
    sc1024 = (sc.astype(np.float32) * WSCALE).astype(np.float16)  # [N, G]
    in_maps = []
    for c in range(NCORES):
        sl = slice(c * NSH, (c + 1) * NSH)
        scc = sc1024[sl]  # [NSH, G]
        sbc = np.empty((NKH, 128, NSH), np.float16)
        for j in range(NKH):
            sbc[j, :64] = scc[:, 2 * j]
            sbc[j, 64:] = scc[:, 2 * j + 1]
        in_maps.append(
            {
                "xt8": xt8,
                "xt16": xt16,
                "wpT": np.ascontiguousarray(wp[sl].T),
                "sbc": sbc,
                "bias": np.ascontiguousarray(b[:, sl]),
            }
        )
    return in_maps


def run(x, weight_packed, scales, bias, trace=False, **build_kw):
    nc = _get_nc(**build_kw)
    K8 = build_kw.get("K8", 1024)
    in_maps = _prep_in_maps(x, weight_packed, scales, bias, K8)
    res = run_bass_kernel_spmd(
        nc, in_maps, core_ids=list(range(NCORES)), trace=trace
    )
    out = np.concatenate([r["out"] for r in res.results], axis=1)
    return out.reshape(B, S, N), res


def kernel(x, weight_packed, scales, bias, group_size=128, **_ignored):
    assert int(np.asarray(group_size)) == 128
    out, _ = run(x, weight_packed, scales, bias)
    return out
